# revision 20
# baseline (speedup 1.0000x reference)
"""Deformable conv block (offset conv -> bilinear deform depthwise -> pointwise)
on 8 Trainium2 NeuronCores, data-parallel over batch (2 images per core).

V2 design (vs. baseline): attacks the three measured bottlenecks
(gpsimd gather-prep 1.51ms, DVE combine 1.28ms, f32 PE 0.78ms):
  1. ONE gather index per (tap, pixel): host pre-builds a bf16 "quad" image
     whose element at (y, x) is the contiguous 768-value 2x2 patch
     [img(y,x), img(y,x+1), img(y+1,x), img(y+1,x+1)] x 192 channels.
     Halves Q7 descriptor-gen work and (with bf16) halves DMA bytes.
  2. Combine uses per-partition-scalar DVE ops (pixels on partitions):
     m = (g00*w00 + g01*w01 + g10*w10 + g11*w11) via 1 tensor_scalar_mul +
     3 fused scalar_tensor_tensor; then one bf16 mult by the depthwise tap
     weight. Weights and fields are computed once, in linear pixel order.
  3. Tap-summation and the pixel->channel transpose are fused into PE
     matmuls accumulating in PSUM (lhsT=m, rhs=identity, start/stop over
     the 9 taps). Offset conv + pointwise run in bf16 (4x PE throughput).
"""

import contextlib

import numpy as np
import ml_dtypes

import concourse.bass as bass
import concourse.bacc as bacc
import concourse.tile as tile
from concourse import mybir
from concourse.bass_utils import run_bass_kernel_spmd
from concourse.masks import make_identity

F32 = mybir.dt.float32
BF16 = mybir.dt.bfloat16
I16 = mybir.dt.int16
AF = mybir.AluOpType

B, C, CO, H, W = 16, 192, 384, 64, 64
HW = H * W
K2 = 9
PADG = 4
WG = W + 2 * PADG          # 72 padded rows/cols
NEG = WG * WG              # 5184 quad elements
QE = 4 * C                 # 768 values per quad element
NCORES = 8
BPC = B // NCORES          # 2 images per core
WC = W + 2                 # 66: conv-pad
NPX = 32                   # pixel groups of 128 (partition dim)
NQT = 8                    # quarters of 512 pixels
TQ = NPX // NQT            # 4 pixel-groups per quarter
NI = 128 * TQ              # 512 idxs per gather call

_cache = {}


def _build():
    if "nc" in _cache:
        return _cache["nc"]
    nc = bacc.Bacc("TRN2", target_bir_lowering=False, debug=False)

    xc0 = nc.dram_tensor("xc0", [BPC, 128, WC, WC], BF16, kind="ExternalInput")
    xc1 = nc.dram_tensor("xc1", [BPC, 64, WC, WC], BF16, kind="ExternalInput")
    xq = nc.dram_tensor("xq", [BPC, NEG, QE], BF16, kind="ExternalInput")
    wdwr = nc.dram_tensor("wdwr", [128, K2, C], BF16, kind="ExternalInput")
    woff0 = nc.dram_tensor("woff0", [128, 9, 18], BF16, kind="ExternalInput")
    woff1 = nc.dram_tensor("woff1", [64, 9, 18], BF16, kind="ExternalInput")
    cstT = nc.dram_tensor("cstT", [128, NPX, 18], F32, kind="ExternalInput")
    cst2 = nc.dram_tensor("cst2", [128, NPX, 18], F32, kind="ExternalInput")
    wpw0 = nc.dram_tensor("wpw0", [128, CO], BF16, kind="ExternalInput")
    wpw1 = nc.dram_tensor("wpw1", [64, CO], BF16, kind="ExternalInput")
    out_d = nc.dram_tensor("out", [BPC, CO, HW], F32, kind="ExternalOutput")
    idx_dram = nc.dram_tensor("idx_scratch", [16, NQT, K2, 32], I16)

    with tile.TileContext(nc) as tc:
        with contextlib.ExitStack() as ctx:
            singles = ctx.enter_context(tc.tile_pool(name="singles", bufs=1))
            work = ctx.enter_context(tc.tile_pool(name="work", bufs=1))
            fbuf = ctx.enter_context(tc.tile_pool(name="fbuf", bufs=1))
            gbuf = ctx.enter_context(tc.tile_pool(name="gbuf", bufs=2))
            mbuf = ctx.enter_context(tc.tile_pool(name="mbuf", bufs=8))
            dbuf = ctx.enter_context(tc.tile_pool(name="dbuf", bufs=2))
            obuf = ctx.enter_context(tc.tile_pool(name="obuf", bufs=2))
            ps_off = ctx.enter_context(tc.tile_pool(name="ps_off", bufs=1, space="PSUM"))
            ps_acc = ctx.enter_context(tc.tile_pool(name="ps_acc", bufs=1, space="PSUM"))
            ps_pw = ctx.enter_context(tc.tile_pool(name="ps_pw", bufs=2, space="PSUM"))

            identb = singles.tile([128, 128], BF16)
            make_identity(nc, identb[:, :])
            ident18 = singles.tile([18, 18], F32)
            make_identity(nc, ident18[:, :])
            s_w0 = singles.tile([128, 9, 18], BF16, tag="sw0")
            nc.sync.dma_start(out=s_w0[:, :, :], in_=woff0[:, :, :])
            s_w1 = singles.tile([64, 9, 18], BF16, tag="sw1")
            nc.sync.dma_start(out=s_w1[:, :, :], in_=woff1[:, :, :])
            s_cT = singles.tile([128, NPX, 18], F32, tag="scT")
            nc.sync.dma_start(out=s_cT[:, :, :], in_=cstT[:, :, :])
            s_c2 = singles.tile([128, NPX, 18], F32, tag="sc2")
            nc.sync.dma_start(out=s_c2[:, :, :], in_=cst2[:, :, :])
            s_dw = singles.tile([128, K2, C], BF16, tag="sdw")
            nc.sync.dma_start(out=s_dw[:, :, :], in_=wdwr[:, :, :])
            s_p0 = singles.tile([128, CO], BF16, tag="sp0")
            nc.sync.dma_start(out=s_p0[:, :], in_=wpw0[:, :])
            s_p1 = singles.tile([64, CO], BF16, tag="sp1")
            nc.sync.dma_start(out=s_p1[:, :], in_=wpw1[:, :])

            wgt_b = []
            idxw_b = []
            for b in range(BPC):
                s_x0 = work.tile([128, WC, WC], BF16, tag="x0")
                nc.sync.dma_start(out=s_x0[:, :, :], in_=xc0[b])
                s_x1 = work.tile([64, WC, WC], BF16, tag="x1")
                nc.sync.dma_start(out=s_x1[:, :, :], in_=xc1[b])

                # ---- offset conv (bf16 PE, f32 psum) ----
                off_sb = work.tile([18, HW], F32, tag="off")
                for q in range(8):
                    pch = ps_off.tile([18, 512], F32, tag="offps")
                    mm = 0
                    for s in range(9):
                        dy, dx = s // 3, s % 3
                        for src, wt in ((s_x0, s_w0), (s_x1, s_w1)):
                            nc.tensor.matmul(
                                pch[:, :],
                                wt[:, s, :],
                                src[:, 8 * q + dy:8 * q + dy + 8, dx:dx + 64],
                                start=(mm == 0),
                                stop=(mm == 17),
                            )
                            mm += 1
                    nc.scalar.copy(off_sb[:, 512 * q:512 * (q + 1)], pch[:, :])

                # ---- transpose offsets to pixel-major (linear order) ----
                offT = work.tile([128, NPX, 18], F32, tag="offT")
                for t in range(NPX):
                    ptr = ps_off.tile([128, 18], F32, tag="trp")
                    nc.tensor.transpose(
                        ptr[:, 0:18], off_sb[:, 128 * t:128 * (t + 1)], ident18[:, :]
                    )
                    nc.scalar.copy(offT[:, t, :], ptr[:, 0:18])

                # ---- fields: positions, floor, frac, idx, bilinear weights ----
                pos = fbuf.tile([128, NPX, 18], F32, tag="pos")
                nc.vector.tensor_tensor(pos[:, :, :], offT[:, :, :], s_cT[:, :, :], AF.add)
                nc.vector.tensor_scalar(pos[:, :, :], pos[:, :, :], 130.5, 60.5, AF.min, AF.max)
                fl = fbuf.tile([128, NPX, 18], F32, tag="fl")
                nc.vector.tensor_scalar(fl[:, :, :], pos[:, :, :], 8388608.0, -8388608.0, AF.add, AF.add)
                frac = fbuf.tile([128, NPX, 18], F32, tag="frac")
                nc.vector.tensor_tensor(frac[:, :, :], fl[:, :, :], pos[:, :, :], AF.is_gt)
                nc.vector.tensor_tensor(fl[:, :, :], fl[:, :, :], frac[:, :, :], AF.subtract)
                nc.vector.tensor_tensor(frac[:, :, :], pos[:, :, :], fl[:, :, :], AF.subtract)
                g1 = fbuf.tile([128, NPX, 18], F32, tag="g1")
                nc.vector.tensor_scalar(g1[:, :, :], frac[:, :, :], -1.0, 1.0, AF.mult, AF.add)

                # ---- second (idx-layout) field path: partition P=16*qt+r,
                # group g -> pixel 512*(P//16) + 128*(g//8) + 16*(g%8) + P%16
                offT2 = fbuf.tile([128, NPX, 18], F32, tag="offT2")
                osb_h = off_sb[:, :]
                for g in range(NPX):
                    view = bass.AP(
                        tensor=osb_h.tensor,
                        offset=osb_h.offset + 128 * (g // 8) + 16 * (g % 8),
                        ap=[osb_h.ap[0], [512, 8], [1, 16]],
                    )
                    stg = fbuf.tile([18, 128], F32, tag="stg", bufs=2)
                    nc.vector.tensor_copy(stg[:, :], view)
                    ptr2 = ps_off.tile([128, 18], F32, tag="trp")
                    nc.tensor.transpose(ptr2[:, 0:18], stg[:, :], ident18[:, :])
                    nc.scalar.copy(offT2[:, g, :], ptr2[:, 0:18])
                pos2 = fbuf.tile([128, NPX, 18], F32, tag="pos2")
                nc.vector.tensor_tensor(pos2[:, :, :], offT2[:, :, :], s_c2[:, :, :], AF.add)
                nc.vector.tensor_scalar(pos2[:, :, :], pos2[:, :, :], 130.5, 60.5, AF.min, AF.max)
                fl2 = fbuf.tile([128, NPX, 18], F32, tag="fl2")
                nc.vector.tensor_scalar(fl2[:, :, :], pos2[:, :, :], 8388608.0, -8388608.0, AF.add, AF.add)
                fr2 = fbuf.tile([128, NPX, 18], F32, tag="fr2")
                nc.vector.tensor_tensor(fr2[:, :, :], fl2[:, :, :], pos2[:, :, :], AF.is_gt)
                nc.vector.tensor_tensor(fl2[:, :, :], fl2[:, :, :], fr2[:, :, :], AF.subtract)
                idxf = fbuf.tile([128, K2, NPX], F32, tag="idxf")
                _if = idxf[:, :, :]
                idxf_v = bass.AP(tensor=_if.tensor, offset=_if.offset,
                                 ap=[_if.ap[0], [1, NPX], [NPX, K2]])
                nc.vector.scalar_tensor_tensor(
                    idxf_v, fl2[:, :, 0:9], 72.0, fl2[:, :, 9:18], AF.mult, AF.add
                )
                idx16 = fbuf.tile([128, K2, NPX], I16, tag="idx16")
                nc.vector.tensor_scalar(idx16[:, :, :], idxf[:, :, :], -4380.0, None, AF.add)

                # difference-basis weights wgt[p, k, t, j] = [fx, fy, fx*fy]
                wgt = fbuf.tile([128, K2, NPX, 4], F32, tag="wgt")
                _w = wgt[:, :, :, :]

                def wv(j):
                    return bass.AP(tensor=_w.tensor, offset=_w.offset + j,
                                   ap=[_w.ap[0], [4, NPX], [NPX * 4, K2]])

                nc.vector.tensor_copy(wv(0), frac[:, :, 9:18])
                nc.vector.tensor_copy(wv(1), frac[:, :, 0:9])
                nc.vector.tensor_tensor(wv(2), frac[:, :, 0:9], frac[:, :, 9:18], AF.mult)
                wgtb = fbuf.tile([128, K2, NPX, 4], BF16, tag="wgtb", bufs=2)
                nc.vector.tensor_copy(wgtb[:, :, :, :], wgt[:, :, :, :])

                # ---- idx 16-partition wrap via DRAM bounce (one DMA each way)
                base = idx_dram[:, :, :, :]
                for qq in range(NQT):
                    wrap_out = bass.AP(
                        tensor=base.tensor,
                        offset=base.offset + 288 * qq,
                        ap=[[2304, 16], [32, K2], [1, 32]],
                    )
                    nc.sync.dma_start(out=wrap_out, in_=idx16[16 * qq:16 * (qq + 1), :, :])
                idxw = fbuf.tile([128, NQT, K2, 32], I16, tag="idxw", bufs=2)
                rep_in = bass.AP(
                    tensor=base.tensor,
                    offset=base.offset,
                    ap=[[0, 8], [2304, 16], [1, 2304]],
                )
                nc.sync.dma_start(out=idxw[:, :, :, :], in_=rep_in)
                wgt_b.append(wgtb)
                idxw_b.append(idxw)

            for b in range(BPC):
                wgt = wgt_b[b]
                idxw = idxw_b[b]
                # ---- gather + combine + fused tap-sum/transpose ----
                xq_b = xq[b]
                for qt in range(NQT):
                    # gather taps in pairs from the single image
                    gk = {}
                    for kp in range(5):
                        k0 = 2 * kp
                        nk = 2 if k0 + 1 < K2 else 1
                        g = gbuf.tile([128, nk * TQ, QE], BF16, tag=f"gp{kp}")
                        src = bass.AP(
                            tensor=xq_b.tensor,
                            offset=xq_b.offset,
                            ap=[[QE, 5111], [1, QE]],
                        )
                        nc.gpsimd.dma_gather(
                            out_ap=g[:, :, :],
                            in_ap=src,
                            idxs_ap=idxw[:, qt, k0:k0 + nk, :],
                            num_idxs=nk * NI,
                            num_idxs_reg=nk * NI,
                            elem_size=QE,
                            elem_step=QE,
                        )
                        for kk in range(nk):
                            gk[k0 + kk] = (g, kk * TQ)

                    # psum accumulators (c-major out); per t128 the 9 taps
                    # accumulate back-to-back (sequential groups per bank)
                    pa = ps_acc.tile([128, 512], F32, tag="pa")
                    pb = ps_acc.tile([64, 512], F32, tag="pb")
                    for t in range(TQ):
                        pg = TQ * qt + t
                        col = 128 * t
                        for kp in range(5):
                            ks = [2 * kp] if 2 * kp + 1 >= K2 else [2 * kp, 2 * kp + 1]
                            ms = []
                            for k in ks:
                                g, ts = gk[k]
                                tt = ts + t
                                m = mbuf.tile([128, C], BF16, tag="m")
                                ms.append((k, g, tt, m))
                            # interleave the two dependent chains to hide
                            # DVE write-to-read latency
                            for step in range(3):
                                for k, g, tt, m in ms:
                                    base_in = (g[:, tt, 0:C] if step == 0
                                               else m[:, :])
                                    nc.vector.scalar_tensor_tensor(
                                        m[:, :],
                                        g[:, tt, (step + 1) * C:(step + 2) * C],
                                        wgt[:, k, pg, step:step + 1],
                                        base_in, AF.mult, AF.add)
                            for k, g, tt, m in ms:
                                m2 = mbuf.tile([128, C], BF16, tag="m2")
                                nc.vector.tensor_tensor(
                                    m2[:, :], m[:, :], s_dw[:, k, :], AF.mult)
                                nc.tensor.matmul(
                                    pa[:, col:col + 128], m2[:, 0:128], identb[:, :],
                                    start=(k == 0), stop=(k == K2 - 1))
                                nc.tensor.matmul(
                                    pb[:, col:col + 128], m2[:, 128:C], identb[:, :],
                                    start=(k == 0), stop=(k == K2 - 1))

                    # copy c-major depthwise result to SBUF (bf16)
                    dwc0 = dbuf.tile([128, 512], BF16, tag="dwc0")
                    dwc1 = dbuf.tile([64, 512], BF16, tag="dwc1")
                    nc.scalar.copy(dwc0[:, :], pa[:, :])
                    nc.scalar.copy(dwc1[:, :], pb[:, :])

                    # ---- pointwise conv for this quarter ----
                    px0 = 512 * qt
                    for o in range(3):
                        ppw = ps_pw.tile([128, 512], F32, tag="pw")
                        nc.tensor.matmul(
                            ppw[:, :],
                            s_p0[:, 128 * o:128 * (o + 1)],
                            dwc0[:, :],
                            start=True, stop=False)
                        nc.tensor.matmul(
                            ppw[:, :],
                            s_p1[:, 128 * o:128 * (o + 1)],
                            dwc1[:, :],
                            start=False, stop=True)
                        osb = obuf.tile([128, 512], F32, tag="osb")
                        nc.scalar.copy(osb[:, :], ppw[:, :])
                        nc.sync.dma_start(
                            out=out_d[b, 128 * o:128 * (o + 1), px0:px0 + 512],
                            in_=osb[:, :])

    nc.compile()
    _cache["nc"] = nc
    return nc


def _host_prep(x, w_off, b_off, w_dw, w_pw):
    K = 3
    bf = ml_dtypes.bfloat16
    # conv input, zero-padded by 1, c-major
    xcp = np.zeros((B, C, WC, WC), np.float32)
    xcp[:, :, 1:65, 1:65] = x
    xcp = xcp.astype(bf)
    # quad gather image: [B, NEG, 768] where elem (y,x) is the 2x2 patch
    xg = np.zeros((B, WG + 1, WG + 1, C), np.float32)
    xg[:, PADG:PADG + H, PADG:PADG + W, :] = np.transpose(x, (0, 2, 3, 1))
    q00 = xg[:, :-1, :-1]
    d01 = xg[:, :-1, 1:] - q00
    d10 = xg[:, 1:, :-1] - q00
    dxy = (xg[:, 1:, 1:] - xg[:, 1:, :-1]) - (xg[:, :-1, 1:] - q00)
    xq = np.concatenate([q00, d01, d10, dxy], axis=-1).reshape(B, NEG, QE).astype(bf)
    wdw = w_dw.reshape(C, K2)
    wdwr = np.broadcast_to(wdw.T[None, :, :], (128, K2, C)).copy().astype(bf)

    # offset conv stationaries, output channels reordered to [y taps | x taps]
    perm = [2 * k for k in range(K2)] + [2 * k + 1 for k in range(K2)]
    wo = np.empty((9, C, 18), np.float32)
    for s in range(9):
        dy, dx = s // 3, s % 3
        wo[s] = w_off[perm, :, dy, dx].T  # [C, 18]
    wo = wo.astype(bf)

    # pixel-major const: pos64 = base + ki/kj - 1 + b_off + 64
    i = np.arange(HW)
    hh, ww = i // W, i % W
    cst = np.empty((HW, 18), np.float32)
    for k in range(K2):
        ki, kj = k // K, k % K
        cst[:, k] = hh - 1 + ki + b_off[2 * k] + 64.0
        cst[:, 9 + k] = ww - 1 + kj + b_off[2 * k + 1] + 64.0
    cstT = cst.reshape(NPX, 128, 18).transpose(1, 0, 2).copy()  # [128, NPX, 18]
    P = np.arange(128)
    G = np.arange(NPX)
    pix2 = (512 * (P[:, None] // 16) + 128 * (G[None, :] // 8)
            + 16 * (G[None, :] % 8) + P[:, None] % 16)  # [128, NPX]
    cst2 = cst[pix2]  # [128, NPX, 18]

    wpwT = w_pw.T.astype(bf)  # [C, CO]

    shared = {
        "woff0": np.ascontiguousarray(wo.transpose(1, 0, 2)[:128]),
        "woff1": np.ascontiguousarray(wo.transpose(1, 0, 2)[128:]),
        "cstT": cstT.astype(np.float32),
        "cst2": cst2.astype(np.float32),
        "wdwr": wdwr,
        "wpw0": np.ascontiguousarray(wpwT[:128]),
        "wpw1": np.ascontiguousarray(wpwT[128:]),
    }
    in_maps = []
    for cid in range(NCORES):
        bs = slice(cid * BPC, (cid + 1) * BPC)
        m = dict(shared)
        m["xc0"] = np.ascontiguousarray(xcp[bs, :128])
        m["xc1"] = np.ascontiguousarray(xcp[bs, 128:])
        m["xq"] = xq[bs]
        in_maps.append(m)
    return in_maps


def kernel(x, w_off, b_off, w_dw, w_pw, _trace=False):
    x = np.asarray(x, np.float32)
    w_off = np.asarray(w_off, np.float32)
    b_off = np.asarray(b_off, np.float32)
    w_dw = np.asarray(w_dw, np.float32)
    w_pw = np.asarray(w_pw, np.float32)

    nc = _build()
    in_maps = _host_prep(x, w_off, b_off, w_dw, w_pw)
    res = run_bass_kernel_spmd(nc, in_maps, core_ids=list(range(NCORES)), trace=_trace)
    out = np.concatenate([r["out"] for r in res.results], axis=0)
    if _trace:
        kernel.last_exec_ns = res.exec_time_ns
    return out.reshape(B, CO, H, W)
